# revision 24
# baseline (speedup 1.0000x reference)
"""Trainium2 Bass kernel for nn_Contrastive_D (contrastive + mapper/entropy loss).

Self-contained: hardcodes shapes from the problem spec.
  b, z: [8192, 128] f32; y: [8192] int; W1 [16,8,256]; b1 [16,256];
  W2 [16,256,256]; b2 [16,256]; perm_idx [128]; flip masks [8192,128] bool.
Returns the scalar loss (f32).

Strategy (8 NeuronCores, SPMD, batch-sharded 1024 rows/core):
  - contrastive path: per-core row-block of logits = bT_rot.T @ bT_rot with the
    full b replicated but COLUMN-ROTATED per core so each core's own diagonal
    block lands at fixed columns (keeps the single SPMD program core-agnostic).
    Streaming masked logsumexp per row (DVE max + ACT exp-accum), target logit
    via host-computed first-same-class index.
  - mapper path: grouped MLP as block-diagonal matmuls in feature-major layout,
    SiLU on ACT, per-(row,subnet) softmax stats with fused accumulate reduces
    (sum-exp on DVE, target select on GpSimd), per-class segment sums via
    one-hot matmul with an extra all-ones column that yields the global mean
    row for free; AllReduced across cores, entropies computed batched on the
    [101, 4096] reduced tensor.
  - host only does: transposes/casts/sharding, index bookkeeping from y,
    and the final sum of 8 partial scalars.
"""

import numpy as np
import ml_dtypes

import concourse.bass as bass
import concourse.bacc as bacc
import concourse.mybir as mybir
import concourse.tile as tile
from concourse.bass_utils import run_bass_kernel_spmd

F32 = mybir.dt.float32
BF16 = mybir.dt.bfloat16
AF = mybir.ActivationFunctionType
ALU = mybir.AluOpType
AX = mybir.AxisListType

NCORES = 8
N = 8192
BITS = 128
M = 16
C = 100
CP1 = C + 1              # extra all-ones column -> global-mean row
TEMP = 0.07
LAM = 0.5
ITEMP = 1.0 / TEMP
R = N // NCORES          # rows per core
NT = R // 128            # 128-row tiles per core
NEG_BIG = 65536.0        # diag mask subtrahend

_CACHE = {}

bf16 = ml_dtypes.bfloat16

# CoreSim doesn't implement the Silu ACT function; sim tests flip this to
# build an equivalent sigmoid+multiply variant (hardware uses native Silu).
SILU_VIA_SIGMOID = False


def _build_program(has_b2=False):
    key = ("nc", has_b2, SILU_VIA_SIGMOID)
    if key in _CACHE:
        return _CACHE[key]
    nc = bacc.Bacc(
        "TRN2", target_bir_lowering=False, debug=False, num_devices=NCORES
    )

    def inp(name, shape, dtype):
        return nc.dram_tensor(name, shape, dtype, kind="ExternalInput")

    zpT = inp("zpT", [128, R], BF16)           # z[:, perm].T shard
    mmT = inp("mmT", [128, R], BF16)           # mapper flip mask (0/1), transposed
    ooT = inp("ooT", [128, R], BF16)           # outer flip mask (0/1), transposed
    pw2 = inp("pow2", [128, 16], BF16)         # block-diag bit weights
    idn = inp("ident", [128, 128], BF16)       # identity
    b1c = inp("b1c", [128, 32], F32)           # layer-1 bias, feature-major blocks
    W1bd = inp("W1bd", [128, 4096], BF16)      # block-diag layer-1 weights
    btT = inp("btT", [128, R], BF16)           # b[t].T shard (un-rotated cols)
    bT = inp("bTrot", [128, N], BF16)          # b.T, columns rotated per core
    W2s = inp("W2s", [128, 32 * 256], BF16)    # layer-2 weights, [k, (2m+hc)*256+o]
    Yb = inp("Yb", [R, CP1], BF16)             # one-hot(y) shard + ones col
    iot = inp("iota", [128, 256], BF16)        # each row 0..255
    bgI = inp("bigI", [128, 128], BF16)        # NEG_BIG * identity
    ivc = inp("invc", [128, 1], F32)           # 1/clip(counts,1); [100] = 1/N
    wvc = inp("wvec", [128, 1], F32)           # LAM*(counts>0); [100] = -1
    onf = inp("ones_f", [128, 1], F32)
    if has_b2:
        b2f = inp("b2f", [128, 4096], F32)     # b2 replicated across partitions

    outv = nc.dram_tensor("outv", [1, 8], F32, kind="ExternalOutput")

    with tile.TileContext(nc) as tc:
        with (
            tc.tile_pool(name="cst", bufs=1) as cst,
            tc.tile_pool(name="dram", bufs=1, space="DRAM") as dram,
        ):
            # ---------------- constant / persistent SBUF ----------------
            # DMA issue order tracks consumption order so compute starts
            # while the big tensors stream in.
            zp_sb = cst.tile([128, R], BF16)
            nc.sync.dma_start(zp_sb[:], zpT[:])
            mm_sb = cst.tile([128, R], BF16)
            nc.sync.dma_start(mm_sb[:], mmT[:])
            oo_sb = cst.tile([128, R], BF16)
            nc.sync.dma_start(oo_sb[:], ooT[:])
            p2_sb = cst.tile([128, 16], BF16)
            nc.sync.dma_start(p2_sb[:], pw2[:])
            id_sb = cst.tile([128, 128], BF16)
            nc.sync.dma_start(id_sb[:], idn[:])
            b1_sb = cst.tile([128, 32], F32)
            nc.sync.dma_start(b1_sb[:], b1c[:])
            W1_sb = cst.tile([128, 4096], BF16)
            for ch in range(4):
                nc.sync.dma_start(
                    W1_sb[:, ch * 1024:(ch + 1) * 1024],
                    W1bd[:, ch * 1024:(ch + 1) * 1024],
                )
            bt_sb = cst.tile([128, R], BF16)
            nc.sync.dma_start(bt_sb[:], btT[:])
            bT_sb = cst.tile([128, N], BF16)
            for ch in range(N // 1024):
                nc.sync.dma_start(
                    bT_sb[:, ch * 1024:(ch + 1) * 1024],
                    bT[:, ch * 1024:(ch + 1) * 1024],
                )
            W2_sb = cst.tile([128, 32 * 256], BF16)
            for ch in range(8):
                nc.sync.dma_start(
                    W2_sb[:, ch * 1024:(ch + 1) * 1024],
                    W2s[:, ch * 1024:(ch + 1) * 1024],
                )
            Y_sb = cst.tile([128, NT * CP1], BF16)
            for t in range(NT):
                nc.sync.dma_start(
                    Y_sb[:, t * CP1:(t + 1) * CP1], Yb[t * 128:(t + 1) * 128, :]
                )
            io_sb = cst.tile([128, 256], BF16)
            nc.sync.dma_start(io_sb[:], iot[:])
            bI_sb = cst.tile([128, 128], BF16)
            nc.sync.dma_start(bI_sb[:], bgI[:])
            ic_sb = cst.tile([128, 1], F32)
            nc.sync.dma_start(ic_sb[:], ivc[:])
            wv_sb = cst.tile([128, 1], F32)
            nc.sync.dma_start(wv_sb[:], wvc[:])
            of_sb = cst.tile([128, 1], F32)
            nc.sync.dma_start(of_sb[:], onf[:])
            if has_b2:
                b2_sb = cst.tile([128, 4096], F32)
                for ch in range(4):
                    nc.sync.dma_start(
                        b2_sb[:, ch * 1024:(ch + 1) * 1024],
                        b2f[:, ch * 1024:(ch + 1) * 1024],
                    )

            sgr = cst.tile([CP1, 4096], BF16)         # all-reduced seg sums
            zf_sb = cst.tile([128, R], BF16)          # zflipT
            hT_sb = cst.tile([128, 32 * R], BF16)     # silu activations, feat-major
            tgt_sb = cst.tile([128, NT * 16], F32)    # per-row byte targets
            pb_sb = cst.tile([128, NT], F32)          # target-logit dot partials
            svA = cst.tile([128, NT * 16], F32)       # per-(row,m) sum-exp
            jsA = cst.tile([128, NT * 16], F32)       # per-(row,m) exp(dec[tgt])
            gmxA = cst.tile([128, NT * 8], F32)       # per-(tile,group) row maxes
            sg4A = cst.tile([128, NT * 8], F32)       # per-(tile,group) sum-exp
            Mcols = cst.tile([128, NT], F32)          # per-tile row maxes
            Scols = cst.tile([128, NT], F32)          # per-tile row sum-exp
            out_sb = cst.tile([1, 8], F32)
            nc.vector.memset(out_sb[:], 0.0)

            seg_part = [
                dram.tile([CP1, 1024], BF16, name=f"seg_part{q}") for q in range(4)
            ]
            seg_red = [
                dram.tile([CP1, 1024], BF16, name=f"seg_red{q}", addr_space="Shared")
                for q in range(4)
            ]

            # ---------------- phase 0: mapper prep ----------------
            with (
                tc.tile_pool(name="w0", bufs=2) as w0,
                tc.tile_pool(name="ps0", bufs=2, space="PSUM") as ps0,
            ):
                sgn = w0.tile([128, R], BF16, tag="sgn")
                nc.vector.tensor_scalar(sgn[:], mm_sb[:], -2.0, 1.0, ALU.mult, ALU.add)
                nc.vector.tensor_tensor(zf_sb[:], sgn[:], zp_sb[:], ALU.mult)
                bit = w0.tile([128, R], BF16, tag="bit")
                nc.vector.tensor_scalar(bit[:], zp_sb[:], 0.0, None, ALU.is_gt)
                nc.vector.tensor_tensor(bit[:], bit[:], oo_sb[:], ALU.not_equal)
                tgT_ps = ps0.tile([16, R], F32, tag="tgT", bufs=1)
                for k in range(R // 512):
                    nc.tensor.matmul(
                        tgT_ps[:, k * 512:(k + 1) * 512],
                        lhsT=p2_sb[:],
                        rhs=bit[:, k * 512:(k + 1) * 512],
                        start=True,
                        stop=True,
                    )
                tgT_sb = w0.tile([16, R], BF16, tag="tgTsb")
                nc.vector.tensor_copy(tgT_sb[:], tgT_ps[:])
                for t in range(NT):
                    tp = ps0.tile([128, 16], BF16, tag="tp", bufs=1)
                    nc.tensor.transpose(
                        tp[:], tgT_sb[0:16, t * 128:(t + 1) * 128], id_sb[0:16, 0:16]
                    )
                    nc.vector.tensor_copy(tgt_sb[:, t * 16:(t + 1) * 16], tp[:])
                # target-logit dots: fused mult+reduce on DVE
                for t in range(NT):
                    jp = w0.tile([128, 128], BF16, tag="jp")
                    nc.vector.tensor_tensor(
                        jp[:],
                        bT_sb[:, t * 128:(t + 1) * 128],
                        bt_sb[:, t * 128:(t + 1) * 128],
                        ALU.mult,
                    )
                    nc.vector.tensor_reduce(
                        pb_sb[:, t:t + 1], jp[:], AX.X, ALU.add
                    )
                # ---------------- phase A: mapper layer 1 ----------------
                for ob in range(32):
                    hp = ps0.tile([128, R], F32, tag="hp")
                    for k in range(R // 512):
                        nc.tensor.matmul(
                            hp[:, k * 512:(k + 1) * 512],
                            lhsT=W1_sb[:, ob * 128:(ob + 1) * 128],
                            rhs=zf_sb[:, k * 512:(k + 1) * 512],
                            start=True,
                            stop=True,
                        )
                    if SILU_VIA_SIGMOID:
                        sig = w0.tile([128, R], BF16, tag="sig")
                        nc.scalar.activation(
                            sig[:], hp[:], AF.Sigmoid,
                            bias=b1_sb[:, ob:ob + 1], scale=1.0,
                        )
                        nc.vector.scalar_tensor_tensor(
                            hT_sb[:, ob * R:(ob + 1) * R],
                            hp[:], b1_sb[:, ob:ob + 1], sig[:],
                            ALU.add, ALU.mult,
                        )
                    else:
                        nc.scalar.activation(
                            hT_sb[:, ob * R:(ob + 1) * R],
                            hp[:],
                            AF.Silu,
                            bias=b1_sb[:, ob:ob + 1],
                            scale=1.0,
                        )

            # ---------------- phase B: mapper layer 2 + seg sums ----------------
            with (
                tc.tile_pool(name="wB", bufs=2) as wB,
                tc.tile_pool(name="psB", bufs=2, space="PSUM") as psB,
                tc.tile_pool(name="psS", bufs=2, space="PSUM") as psS,
            ):
                for q in range(4):
                    seg_ps = psS.tile([CP1, 1024], F32, tag="seg")
                    for t in range(NT):
                        dec_ps = psB.tile([128, 1024], F32, tag="dec")
                        if has_b2:
                            nc.vector.tensor_copy(
                                dec_ps[:], b2_sb[:, q * 1024:(q + 1) * 1024]
                            )
                        for mq in range(4):
                            m = 4 * q + mq
                            for hc in range(2):
                                fb = 2 * m + hc
                                nc.tensor.matmul(
                                    dec_ps[:, mq * 256:(mq + 1) * 256],
                                    lhsT=hT_sb[:, fb * R + t * 128:fb * R + (t + 1) * 128],
                                    rhs=W2_sb[:, fb * 256:(fb + 1) * 256],
                                    start=(hc == 0 and not has_b2),
                                    stop=(hc == 1),
                                    skip_group_check=True,
                                )
                        esb = wB.tile([128, 1024], BF16, tag="esb", bufs=3)
                        nc.scalar.activation(esb[:], dec_ps[:], AF.Exp)
                        dcb = wB.tile([128, 1024], BF16, tag="dcb", bufs=3)
                        # DVE is the phase bottleneck: evacuate on ACT
                        nc.scalar.copy(dcb[:], dec_ps[:])
                        nc.vector.tensor_reduce(
                            svA[:, t * 16 + 4 * q:t * 16 + 4 * q + 4],
                            esb[:].rearrange("p (m o) -> p m o", o=256),
                            AX.X,
                            ALU.add,
                        )
                        for mq in range(4):
                            m = 4 * q + mq
                            jscr = wB.tile([128, 256], BF16, tag="jscr", bufs=2)
                            nc.vector.scalar_tensor_tensor(
                                jscr[:],
                                io_sb[:],
                                tgt_sb[:, t * 16 + m:t * 16 + m + 1],
                                esb[:, mq * 256:(mq + 1) * 256],
                                ALU.is_equal,
                                ALU.mult,
                                accum_out=jsA[:, t * 16 + m:t * 16 + m + 1],
                            )
                        for h in range(2):
                            nc.tensor.matmul(
                                seg_ps[:, h * 512:(h + 1) * 512],
                                lhsT=Y_sb[:, t * CP1:(t + 1) * CP1],
                                rhs=dcb[:, h * 512:(h + 1) * 512],
                                start=(t == 0),
                                stop=(t == NT - 1),
                                skip_group_check=True,
                            )
                    sg_sb = wB.tile([CP1, 1024], BF16, tag="sgev")
                    nc.vector.tensor_copy(sg_sb[:], seg_ps[:])
                    nc.sync.dma_start(seg_part[q][:], sg_sb[:])
                    # per-chunk AllReduce: starts as soon as this q-chunk is
                    # written, pipelining the collective under phases B/C
                    nc.gpsimd.collective_compute(
                        "AllReduce",
                        ALU.add,
                        replica_groups=[list(range(NCORES))],
                        ins=[seg_part[q].opt()],
                        outs=[seg_red[q].opt()],
                    )
                    nc.sync.dma_start(
                        sgr[:, q * 1024:(q + 1) * 1024], seg_red[q][:]
                    )

            # ---------------- phase C: contrastive logits ----------------
            with (
                tc.tile_pool(name="wC", bufs=2) as wC,
                tc.tile_pool(name="psC", bufs=2, space="PSUM") as psC,
            ):
                NG = 8
                for t in range(NT):
                    for g in range(NG):
                        lg = psC.tile([128, 1024], F32, tag="lg", bufs=4)
                        for k in range(2):
                            nc.tensor.matmul(
                                lg[:, k * 512:(k + 1) * 512],
                                lhsT=bT_sb[:, t * 128:(t + 1) * 128],
                                rhs=bT_sb[:, g * 1024 + k * 512:g * 1024 + (k + 1) * 512],
                                start=True,
                                stop=True,
                            )
                        if g == 0:
                            nc.vector.tensor_tensor(
                                lg[:, t * 128:(t + 1) * 128],
                                lg[:, t * 128:(t + 1) * 128],
                                bI_sb[:],
                                ALU.subtract,
                            )
                        gc = t * NG + g
                        # negated max doubles as the exp bias directly
                        nc.vector.tensor_reduce(
                            gmxA[:, gc:gc + 1], lg[:], AX.X, ALU.max, negate=True
                        )
                        je = wC.tile([128, 1024], BF16, tag="je")
                        nc.scalar.activation(
                            je[:],
                            lg[:],
                            AF.Exp,
                            bias=gmxA[:, gc:gc + 1],
                            scale=1.0,
                            accum_out=sg4A[:, gc:gc + 1],
                        )
                    # per-tile combine of the NG groups (all values negated:
                    # Mcols holds -rowmax = min of the negated group maxes)
                    nc.vector.tensor_reduce(
                        Mcols[:, t:t + 1], gmxA[:, t * NG:(t + 1) * NG], AX.X,
                        ALU.min,
                    )
                    wg = wC.tile([128, NG], F32, tag="wg")
                    nc.scalar.activation(
                        wg[:], gmxA[:, t * NG:(t + 1) * NG], AF.Exp,
                        bias=Mcols[:, t:t + 1], scale=-1.0,
                    )
                    j4 = wC.tile([128, NG], F32, tag="j4")
                    nc.vector.tensor_tensor(
                        j4[:], sg4A[:, t * NG:(t + 1) * NG], wg[:], ALU.mult
                    )
                    nc.vector.tensor_reduce(
                        Scols[:, t:t + 1], j4[:], AX.X, ALU.add
                    )

            # ---------------- final combine + entropy ----------------
            with (
                tc.tile_pool(name="wE", bufs=1) as wE,
                tc.tile_pool(name="psE", bufs=1, space="PSUM") as psE,
            ):
                # entropy over all-reduced [CP1, 4096] segment sums; row C is
                # the N-weighted global mean (inter entropy) via the ones col.
                # Chunked so each 1024-col chunk processes as its AllReduce
                # lands, overlapping the contrastive phase.
                SmE = wE.tile([CP1, 16], F32, tag="SmE")
                T1E = wE.tile([CP1, 16], F32, tag="T1E")
                for ch in range(4):
                    cs = slice(ch * 1024, (ch + 1) * 1024)
                    mnsb = wE.tile([CP1, 1024], BF16, tag="mnsb", bufs=2)
                    nc.vector.tensor_scalar(
                        mnsb[:], sgr[:, cs], ic_sb[0:CP1, 0:1], None, ALU.mult
                    )
                    eEb = wE.tile([CP1, 1024], BF16, tag="eEb", bufs=2)
                    nc.scalar.activation(eEb[:], mnsb[:], AF.Exp)
                    nc.vector.tensor_reduce(
                        SmE[:, ch * 4:(ch + 1) * 4],
                        eEb[:].rearrange("p (m o) -> p m o", o=256),
                        AX.X,
                        ALU.add,
                    )
                    pEb = wE.tile([CP1, 1024], BF16, tag="pEb", bufs=2)
                    nc.vector.tensor_tensor(pEb[:], eEb[:], mnsb[:], ALU.mult)
                    nc.vector.tensor_reduce(
                        T1E[:, ch * 4:(ch + 1) * 4],
                        pEb[:].rearrange("p (m o) -> p m o", o=256),
                        AX.X,
                        ALU.add,
                    )
                # H = ln(S) - T1/S; net = sum_m sum_c wvec_c * H[c, m]
                siE = wE.tile([128, 16], F32, tag="siE")
                nc.vector.reciprocal(siE[0:CP1, :], SmE[:])
                lsE = wE.tile([128, 16], F32, tag="lsE")
                nc.scalar.activation(lsE[0:CP1, :], SmE[:], AF.Ln)
                tE = wE.tile([128, 16], F32, tag="tE")
                nc.vector.tensor_tensor(tE[0:CP1, :], T1E[:], siE[0:CP1, :], ALU.mult)
                hE = wE.tile([128, 16], F32, tag="hE")
                nc.vector.tensor_tensor(hE[0:CP1, :], lsE[0:CP1, :], tE[0:CP1, :], ALU.subtract)
                net_ps = psE.tile([1, 16], F32, tag="net", bufs=1)
                nc.tensor.matmul(
                    net_ps[:], lhsT=wv_sb[0:CP1, 0:1], rhs=hE[0:CP1, :],
                    start=True, stop=True,
                )
                nc.vector.tensor_reduce(out_sb[0:1, 2:3], net_ps[:], AX.X, ALU.add)

                # mapLoss per-row partials: sum_m [ln(sum-exp) - ln(exp(dec[tgt]))]
                lnsv = wE.tile([128, NT * 16], F32, tag="lnsv")
                nc.scalar.activation(lnsv[:], svA[:], AF.Ln)
                lnjs = wE.tile([128, NT * 16], F32, tag="lnjs")
                nc.scalar.activation(lnjs[:], jsA[:], AF.Ln)
                jm = wE.tile([128, NT * 16], F32, tag="jm")
                nc.vector.tensor_tensor(jm[:], lnsv[:], lnjs[:], ALU.subtract)
                mlsr = wE.tile([128, 1], F32, tag="mlsr")
                nc.vector.tensor_reduce(mlsr[:], jm[:], AX.X, ALU.add)

                # contrastive per-row: lse = -Mcols + ln(Scols); base part
                # (temperature already folded into the matmul inputs)
                lnS = wE.tile([128, NT], F32, tag="lnS")
                nc.scalar.activation(lnS[:], Scols[:], AF.Ln)
                bc = wE.tile([128, NT], F32, tag="bc")
                nc.vector.scalar_tensor_tensor(
                    bc[:], Mcols[:], -1.0, lnS[:], ALU.mult, ALU.add
                )
                rr = wE.tile([128, 4], F32, tag="rr")
                nc.vector.tensor_reduce(rr[:, 0:1], bc[:], AX.X, ALU.add)
                nc.vector.tensor_reduce(rr[:, 1:2], pb_sb[:], AX.X, ALU.add)
                cmb = wE.tile([128, 2], F32, tag="cmb")
                nc.vector.scalar_tensor_tensor(
                    cmb[:, 0:1], rr[:, 1:2], -1.0, rr[:, 0:1], ALU.mult, ALU.add
                )
                nc.vector.tensor_copy(cmb[:, 1:2], mlsr[:])
                fin_ps = psE.tile([1, 2], F32, tag="fin", bufs=1)
                nc.tensor.matmul(fin_ps[:], lhsT=of_sb[:], rhs=cmb[:], start=True, stop=True)
                nc.vector.tensor_copy(out_sb[:, 0:2], fin_ps[:])
                nc.sync.dma_start(outv[:], out_sb[:])

    nc.finalize()
    _CACHE[key] = nc
    return nc


def _host_prep(b, z, y, W1, b1, W2, b2, perm_idx, flip_mask_mapper, flip_mask_outer):
    """Build the 8 per-core input maps (layout/cast/index work only)."""
    b = np.asarray(b, np.float32)
    z = np.asarray(z, np.float32)
    y = np.asarray(y).astype(np.int64)
    W1 = np.asarray(W1, np.float32)
    b1 = np.asarray(b1, np.float32)
    W2 = np.asarray(W2, np.float32)
    b2 = np.asarray(b2, np.float32)
    perm_idx = np.asarray(perm_idx).astype(np.int64)
    fm = np.asarray(flip_mask_mapper).astype(bool)
    fo = np.asarray(flip_mask_outer).astype(bool)
    has_b2 = bool(np.any(b2))

    # first-same-class target index per row
    first = np.full(C, -1, np.int64)
    second = np.full(C, -1, np.int64)
    for j in range(N):
        c = y[j]
        if first[c] < 0:
            first[c] = j
        elif second[c] < 0:
            second[c] = j
    t_idx = np.empty(N, np.int64)
    for i in range(N):
        f = first[y[i]]
        if f != i:
            t_idx[i] = f
        elif second[y[i]] >= 0:
            t_idx[i] = second[y[i]]
        else:
            t_idx[i] = 1 if i == 0 else 0

    # fold the softmax temperature into b: logits come out of the matmul
    # already scaled by 1/TEMP, so no scale factors are needed on-device
    cs = np.sqrt(ITEMP)
    bT = np.ascontiguousarray(b.T * cs).astype(bf16)          # [128, N]
    btT = np.ascontiguousarray(b[t_idx].T * cs).astype(bf16)  # [128, N]
    zpT = np.ascontiguousarray(z[:, perm_idx].T).astype(bf16)
    mmT = np.ascontiguousarray(fm.T).astype(bf16)
    ooT = np.ascontiguousarray(fo.T).astype(bf16)
    Y = np.zeros((N, CP1), bf16)
    Y[np.arange(N), y] = 1
    Y[:, C] = 1

    W1bd = np.zeros((128, 4096), np.float32)
    for m in range(M):
        W1bd[8 * m:8 * m + 8, 256 * m:256 * m + 256] = W1[m]
    W1bd = W1bd.astype(bf16)
    W2s = np.zeros((128, 32 * 256), np.float32)
    for m in range(M):
        for hc in range(2):
            W2s[:, (2 * m + hc) * 256:(2 * m + hc + 1) * 256] = W2[m, hc * 128:(hc + 1) * 128, :]
    W2s = W2s.astype(bf16)
    b1c = np.ascontiguousarray(b1.reshape(4096).reshape(32, 128).T).astype(np.float32)
    iota = np.broadcast_to(np.arange(256, dtype=np.float32), (128, 256)).astype(bf16)
    pow2 = np.zeros((128, 16), np.float32)
    for m in range(M):
        pow2[8 * m:8 * m + 8, m] = 2.0 ** np.arange(8)
    pow2 = pow2.astype(bf16)
    ident = np.eye(128, dtype=np.float32).astype(bf16)
    bigI = (NEG_BIG * np.eye(128, dtype=np.float32)).astype(bf16)
    counts = np.bincount(y, minlength=C).astype(np.float32)
    invc = np.zeros((128, 1), np.float32)
    invc[:C, 0] = 1.0 / np.clip(counts, 1.0, None)
    invc[C, 0] = 1.0 / N
    wvec = np.zeros((128, 1), np.float32)
    wvec[:C, 0] = LAM * (counts > 0).astype(np.float32)
    wvec[C, 0] = -1.0
    ones_f = np.ones((128, 1), np.float32)

    in_maps = []
    for core in range(NCORES):
        sl = slice(core * R, (core + 1) * R)
        m_ = dict(
            bTrot=np.ascontiguousarray(np.roll(bT, -core * R, axis=1)),
            btT=np.ascontiguousarray(btT[:, sl]),
            zpT=np.ascontiguousarray(zpT[:, sl]),
            mmT=np.ascontiguousarray(mmT[:, sl]),
            ooT=np.ascontiguousarray(ooT[:, sl]),
            Yb=np.ascontiguousarray(Y[sl]),
            W1bd=W1bd,
            W2s=W2s,
            b1c=b1c,
            iota=iota,
            pow2=pow2,
            ident=ident,
            bigI=bigI,
            invc=invc,
            wvec=wvec,
            ones_f=ones_f,
        )
        if has_b2:
            m_["b2f"] = np.broadcast_to(
                b2.reshape(1, 4096), (128, 4096)
            ).astype(np.float32).copy()
        in_maps.append(m_)
    return in_maps, has_b2


def kernel(**inputs) -> np.ndarray:
    in_maps, has_b2 = _host_prep(**inputs)
    nc = _build_program(has_b2)
    _CACHE["last_in_maps"] = in_maps
    res = run_bass_kernel_spmd(nc, in_maps, list(range(NCORES)))
    _CACHE["last_results"] = res
    outs = [r["outv"] for r in res.results]
    base_sum = sum(float(o[0, 0]) for o in outs)
    mls_sum = sum(float(o[0, 1]) for o in outs)
    net = float(outs[0][0, 2])
    loss = base_sum / N + mls_sum / N + net
    return np.float32(loss)


def measure_hw_ns(n_iter=30):
    """Device-resident repeated execution timing (min wall per call).

    Includes PJRT dispatch overhead, so it is an upper bound on true
    on-device exec time; test.py reports neuron-profile time instead.
    """
    import time
    import jax
    from jax.sharding import Mesh, PartitionSpec, NamedSharding
    from jax.experimental.shard_map import shard_map
    from concourse import bass2jax as b2j
    import concourse.mybir as mybir_

    nc = _build_program(False)
    in_maps = _CACHE["last_in_maps"]
    b2j.install_neuronx_cc_hook()

    partition_name = nc.partition_id_tensor.name if nc.partition_id_tensor else None
    in_names, out_names, out_avals, zero_outs = [], [], [], []
    for alloc in nc.m.functions[0].allocations:
        if not isinstance(alloc, mybir_.MemoryLocationSet):
            continue
        name = alloc.memorylocations[0].name
        if alloc.kind == "ExternalInput":
            if name != partition_name:
                in_names.append(name)
        elif alloc.kind == "ExternalOutput":
            shape = tuple(alloc.tensor_shape)
            np_dt = mybir_.dt.np(alloc.dtype)
            out_names.append(name)
            out_avals.append(jax.core.ShapedArray(shape, np_dt))
            zero_outs.append(np.zeros(shape, np_dt))
    n_params = len(in_names)
    n_outs = len(out_names)
    all_in_names = list(in_names) + list(out_names)
    if partition_name is not None:
        all_in_names.append(partition_name)

    def _body(*args):
        operands = list(args)
        if partition_name is not None:
            operands.append(b2j.partition_id_tensor())
        outs = b2j._bass_exec_p.bind(
            *operands,
            out_avals=tuple(out_avals),
            in_names=tuple(all_in_names),
            out_names=tuple(out_names),
            lowering_input_output_aliases=(),
            sim_require_finite=True,
            sim_require_nnan=True,
            nc=nc,
        )
        return tuple(outs)

    devices = jax.devices()[:NCORES]
    mesh = Mesh(np.asarray(devices), ("core",))
    in_specs = (PartitionSpec("core"),) * (n_params + n_outs)
    out_specs = (PartitionSpec("core"),) * n_outs
    fn = jax.jit(
        shard_map(_body, mesh=mesh, in_specs=in_specs, out_specs=out_specs,
                  check_rep=False),
        keep_unused=True,
    )
    per_core = [[np.asarray(m[name]) for name in in_names] for m in in_maps]
    concat_in = [
        np.concatenate([per_core[c][i] for c in range(NCORES)], axis=0)
        for i in range(n_params)
    ]
    concat_zeros = [
        np.zeros((NCORES * z.shape[0], *z.shape[1:]), z.dtype) for z in zero_outs
    ]
    sh = NamedSharding(mesh, PartitionSpec("core"))
    dev_in = [jax.device_put(a, sh) for a in concat_in]
    dev_zero = [jax.device_put(a, sh) for a in concat_zeros]
    for _ in range(3):
        r = fn(*dev_in, *dev_zero)
        jax.block_until_ready(r)
    times = []
    for _ in range(n_iter):
        t0 = time.perf_counter()
        r = fn(*dev_in, *dev_zero)
        jax.block_until_ready(r)
        times.append(time.perf_counter() - t0)
    times.sort()
    return dict(
        min_ns=int(times[0] * 1e9),
        p50_ns=int(times[len(times) // 2] * 1e9),
        mean_ns=int(sum(times) / len(times) * 1e9),
    )
